# revision 1
# baseline (speedup 1.0000x reference)
"""Trainium2 Bass kernel for nn_AbstractionLayer_87222195847181.

Strategy: batch-parallel over 8 NeuronCores (one batch element per core).
Per core: (1) the sampling scan runs as a single-engine DVE While loop with
2-cycle early exit + alternation fill; (2) grouping scores via K=4 PE matmuls
(score = <c,x> - 0.5*|x|^2, order-equivalent to squared distance), top-32
selection via 16-point windowed max + max8/max_index/match_replace rounds,
candidate windows re-gathered (dma_gather, 256B descs) and re-scored as exact
squared distances on DVE; (3) shared-MLP pointnet on gathered members with the
group-max fused on PSUM; only ceil(distinct/128) centroid blocks are computed
(For_i_w_nested_ifs), remaining rows replicated via a dma_gather row gather.
"""
import os
import numpy as np
import concourse.bass as bass
import concourse.bacc as bacc
import concourse.mybir as mybir
import concourse.tile as tile
from concourse.bass import ds, IndirectOffsetOnAxis
from concourse.bass_utils import run_bass_kernel_spmd
from concourse.masks import make_identity
from ordered_set import OrderedSet

P = 128
NEG = -3.0e38
F32 = mybir.dt.float32
I32 = mybir.dt.int32
U32 = mybir.dt.uint32
U16 = mybir.dt.uint16
N = 16384
M = 512
R = 32
W = 16           # selection window size
NW = N // W      # 1024 windows per row
NCAND = R * W    # 512 candidates per row
NBLK = 4         # max centroid blocks (M/128)
A = mybir.AluOpType
AF = mybir.ActivationFunctionType
NCORES = 8



def alloc_scan_tiles(sb):
    t = {}
    t["acc"] = sb.tile([P, P], F32, name="scan_acc")
    t["cm8"] = sb.tile([P, 8], F32, name="scan_cm8")
    t["ci8"] = sb.tile([P, 8], U32, name="scan_ci8")
    t["pk"] = sb.tile([P, 32], F32, name="scan_pk")
    t["pkT"] = sb.tile([P, 32], F32, name="scan_pkT")
    t["row"] = sb.tile([1, P], F32, name="scan_row")
    t["g8"] = sb.tile([1, 8], F32, name="scan_g8")
    t["gi8"] = sb.tile([1, 8], U32, name="scan_gi8")
    t["tb"] = sb.tile([32, 32], F32, name="scan_tb")
    t["scal"] = sb.tile([P, 32], F32, name="scan_scal")
    t["gmap"] = sb.tile([1, 512], I32, name="scan_gmap")
    t["galt"] = sb.tile([1, 512], I32, name="scan_galt")
    t["jmask"] = sb.tile([1, 512], U32, name="scan_jmask")
    t["gmap16"] = sb.tile([1, 512], U16, name="scan_gmap16")
    t["idxs16"] = sb.tile([P, 32], U16, name="scan_idxs16")
    t["kinfo"] = sb.tile([32, 8], I32, name="scan_kinfo")  # row0: [K*, NB, ...]
    t["neg1"] = sb.tile([1, 512], F32, name="scan_neg1")
    t["ktileT"] = sb.tile([P, 32], I32, name="scan_ktileT")
    t["galtw"] = sb.tile([P, 32], I32, name="scan_galtw")
    t["jmaskw"] = sb.tile([P, 32], U32, name="scan_jmaskw")
    t["gmapw"] = sb.tile([P, 32], I32, name="scan_gmapw")
    t["jrow"] = sb.tile([1, 512], I32, name="scan_jrow")
    t["jroww"] = sb.tile([P, 32], I32, name="scan_jroww")
    t["iotaP"] = sb.tile([P, 1], I32, name="scan_iotaP")
    return t


def emit_scan_setup(nc, t, T4, lhsTg, M):
    """Static presets; call under Tile scheduling (not critical)."""
    A = mybir.AluOpType
    nc.gpsimd.iota(t["jrow"], pattern=[[1, M]], base=0, channel_multiplier=0)
    # jroww[p, s] = (p % 16) + 16*s  (same index table replicated per core)
    nc.gpsimd.iota(t["iotaP"], pattern=[[0, 1]], base=0, channel_multiplier=1)
    nc.gpsimd.iota(t["jroww"], pattern=[[16, 32]], base=0, channel_multiplier=0)
    nc.vector.tensor_scalar(out=t["iotaP"], in0=t["iotaP"], scalar1=15,
                            scalar2=None, op0=A.bitwise_and)
    nc.vector.tensor_tensor(
        out=t["jroww"], in0=t["jroww"],
        in1=t["iotaP"].to_broadcast([P, 32]), op=A.add)
    nc.vector.memset(t["pk"], NEG)
    nc.vector.memset(t["acc"], 0.0)
    nc.vector.memset(t["row"], 0.0)
    nc.vector.memset(t["scal"], 0.0)
    nc.vector.memset(t["tb"], 0.0)
    nc.vector.memset(t["cm8"], 0.0)
    nc.vector.memset(t["ci8"].bitcast(F32), 0.0)
    nc.vector.memset(t["g8"], 0.0)
    nc.vector.memset(t["gi8"].bitcast(F32), 0.0)
    nc.vector.memset(t["gmap"].bitcast(F32), 0.0)
    nc.vector.memset(t["gmap16"].bitcast(mybir.dt.float16), 0.0)
    nc.vector.memset(t["idxs16"].bitcast(mybir.dt.float16), 0.0)
    nc.vector.memset(t["kinfo"].bitcast(F32), 0.0)
    nc.vector.memset(t["ktileT"].bitcast(F32), 0.0)
    nc.vector.memset(t["galtw"].bitcast(F32), 0.0)
    nc.vector.memset(t["jmaskw"].bitcast(F32), 0.0)
    nc.vector.memset(t["gmapw"].bitcast(F32), 0.0)
    # lhsTg: rows 0..2 coords, row 3 = -1 (multiplies xsqhalf), rest 0
    nc.vector.memset(lhsTg, 0.0)
    nc.vector.memset(t["neg1"], -1.0)
    nc.sync.dma_start(out=lhsTg[3:4, :], in_=t["neg1"][:, 0:lhsTg.shape[1]])
    # col 0 = first centroid = point 0
    nc.vector.tensor_copy(out=lhsTg[0:3, 0:1], in_=T4[0:3, 0:1])
    # scal <- broadcast coords of point 0
    nc.vector.transpose(out=t["tb"], in_=T4[0:32, 0:1].to_broadcast([32, 32]))
    for q in range(4):
        nc.vector.tensor_copy(out=t["scal"][32 * q:32 * (q + 1), 0:32], in_=t["tb"])


def emit_scan_loop(nc, t, T4, X2, Y2, Z2, XSQ, lhsTg, M):
    """Raw DVE While loop. Must be inside tc.tile_critical().
    Writes lhsTg cols 1..K*-ish, gmap [1,512] i32, idxs16 [16,32] u16 (wrapped),
    kinfo[0,0]=K*, kinfo[0,1]=NB."""
    from ordered_set import OrderedSet
    V = nc.vector
    ET = mybir.EngineType
    veng = OrderedSet([ET.DVE])

    rN = V.alloc_register("scan_n")
    rF = V.alloc_register("scan_f")
    rN1 = V.alloc_register("scan_n1")
    rN2 = V.alloc_register("scan_n2")
    rK = V.alloc_register("scan_k")
    rGo = V.alloc_register("scan_go")
    rT = V.alloc_register("scan_t")

    V.reg_mov(rN1, 0)
    V.reg_mov(rN2, -1)
    V.reg_mov(rK, 1)
    V.reg_mov(rGo, 1)
    V.reg_mov(rN, 0)
    V.reg_mov(rF, 0)
    V.reg_mov(rT, 0)

    nsv = V.snap(rN, donate=True, min_val=0, max_val=16383)
    ksv = V.snap(rK, donate=True, min_val=0, max_val=M - 1)
    gosv = V.snap(rGo, donate=True, min_val=0, max_val=1)

    with V.While(lambda: gosv & (ksv < M)):
        # score = XSQ + x*(-2lx) + ... via stt chain (scal holds raw coords;
        # X2/Y2/Z2 are -2*coord tiles)
        V.scalar_tensor_tensor(
            out=t["acc"], in0=X2, scalar=t["scal"][:, 0:1], in1=XSQ,
            op0=mybir.AluOpType.mult, op1=mybir.AluOpType.add)
        V.drain()
        V.scalar_tensor_tensor(
            out=t["acc"], in0=Y2, scalar=t["scal"][:, 1:2], in1=t["acc"],
            op0=mybir.AluOpType.mult, op1=mybir.AluOpType.add)
        V.drain()
        V.scalar_tensor_tensor(
            out=t["acc"], in0=Z2, scalar=t["scal"][:, 2:3], in1=t["acc"],
            op0=mybir.AluOpType.mult, op1=mybir.AluOpType.add)
        V.drain()
        V.max(out=t["cm8"], in_=t["acc"])
        V.drain()
        V.max_index(out=t["ci8"], in_max=t["cm8"], in_values=t["acc"])
        pk_i = t["pk"].bitcast(I32)
        V.drain()
        V.tensor_scalar(
            out=pk_i[:, 0:1], in0=t["cm8"][:, 0:1].bitcast(I32),
            scalar1=~127, scalar2=None, op0=mybir.AluOpType.bitwise_and)
        V.drain()
        V.tensor_tensor(
            out=pk_i[:, 0:1], in0=pk_i[:, 0:1],
            in1=t["ci8"][:, 0:1].bitcast(I32), op=mybir.AluOpType.bitwise_or)
        V.drain()
        V.transpose(out=t["pkT"], in_=t["pk"])
        V.drain()
        for q in range(4):
            V.tensor_copy(out=t["row"][0:1, 32 * q:32 * (q + 1)],
                          in_=t["pkT"][32 * q:32 * q + 1, 0:32])
        V.drain()
        V.max(out=t["g8"], in_=t["row"])
        V.drain()
        V.max_index(out=t["gi8"], in_max=t["g8"], in_values=t["row"])
        V.drain()
        # n* = (p* << 7) | (bits(g8[0]) & 127)
        V.reg_load(rF, t["g8"][0:1, 0:1].bitcast(I32))
        V.reg_alu(rF, rF, 127, mybir.AluOpType.bitwise_and)
        V.reg_load(rN, t["gi8"][0:1, 0:1].bitcast(I32))
        V.reg_alu(rN, rN, 7, mybir.AluOpType.logical_shift_left)
        V.reg_alu(rN, rN, rF, mybir.AluOpType.bitwise_or)
        # cycle check vs i_{k-2}
        V.reg_mov(rGo, rN)
        V.reg_alu(rGo, rGo, rN2, mybir.AluOpType.not_equal)
        V.reg_mov(rN2, rN1)
        V.reg_mov(rN1, rN)
        # write lhsTg col k
        V.tensor_copy(out=lhsTg[0:3, ds(ksv, 1)], in_=T4[0:3, ds(nsv, 1)])
        # scal <- broadcast coords of point n*
        V.transpose(out=t["tb"], in_=T4[0:32, ds(nsv, 1)].to_broadcast([32, 32]))
        V.drain()
        for q in range(4):
            V.tensor_copy(out=t["scal"][32 * q:32 * (q + 1), 0:32], in_=t["tb"])
        V.drain()
        V.reg_alu(rK, rK, 1, mybir.AluOpType.add)

    # K* = rK - 1 + rGo;  NB = ceil(K*/128)
    V.reg_alu(rK, rK, 1, mybir.AluOpType.subtract)
    V.reg_alu(rK, rK, rGo, mybir.AluOpType.add)
    V.reg_save(t["kinfo"][0:1, 0:1], ksv)
    V.reg_mov(rT, rK)
    V.reg_alu(rT, rT, 127, mybir.AluOpType.add)
    V.reg_alu(rT, rT, 7, mybir.AluOpType.arith_shift_right)
    tsv = V.snap(rT, donate=True, min_val=0, max_val=4)
    V.reg_save(t["kinfo"][0:1, 1:2], tsv)
    V.drain()

    # g-map: g(j) = j < K* ? j : (K*-2) + ((j-K*)&1), computed twice:
    # linear [1,512] (for output-row gather) and wrapped [16,32] (indirect_copy)
    A = mybir.AluOpType

    def gmap_calc(jr, kc, galt, jmask, gmap):
        V.tensor_tensor(out=galt, in0=jr, in1=kc, op=A.subtract)
        V.drain()
        V.tensor_scalar(out=galt, in0=galt, scalar1=1, scalar2=None,
                        op0=A.bitwise_and)
        V.drain()
        V.tensor_tensor(out=galt, in0=galt, in1=kc, op=A.add)
        V.drain()
        V.tensor_scalar(out=galt, in0=galt, scalar1=-2, scalar2=None, op0=A.add)
        V.tensor_tensor(out=jmask, in0=jr, in1=kc, op=A.is_lt)
        V.drain()
        V.select(out=gmap, mask=jmask, on_true=jr, on_false=galt, add_drain=True)
        V.drain()

    kcol = t["kinfo"][0:1, 0:1].to_broadcast([1, 512])
    gmap_calc(t["jrow"], kcol, t["galt"], t["jmask"], t["gmap"])
    # broadcast K* across partitions: transpose of [32,32] free-broadcast
    V.transpose(out=t["ktileT"].bitcast(F32)[0:32, :],
                in_=t["kinfo"][:, 0:1].bitcast(F32).to_broadcast([32, 32]))
    V.drain()
    for q in range(1, 4):
        V.tensor_copy(out=t["ktileT"].bitcast(F32)[32 * q:32 * (q + 1), 0:1],
                      in_=t["ktileT"].bitcast(F32)[0:32, 0:1])
    V.drain()
    kcolw = t["ktileT"][:, 0:1].to_broadcast([P, 32])
    gmap_calc(t["jroww"], kcolw, t["galtw"], t["jmaskw"], t["gmapw"])
    # u16 wrapped index table for indirect_copy
    V.tensor_copy(out=t["idxs16"], in_=t["gmapw"])
    V.drain()


def emit_scan_gather(nc, t, lhsTg, lhsTgF, sem):
    """After the critical loop: relayout gmap16 -> idxs16 (DMA), then gpsimd
    indirect_copy to gather lhsTg columns into lhsTgF. Caller supplies a raw
    semaphore for the DVE->DMA->gpsimd handoff if used inside a critical;
    under normal Tile scheduling pass sem=None."""
    nc.gpsimd.indirect_copy(
        out=lhsTgF, data=lhsTg, idxs=t["idxs16"],
        i_know_ap_gather_is_preferred=True)



def emit_kernel(tc, nc, sb, psum, dram, ins, out_final, dedup=False,
                scramble=True, level=0, dbg_out=None, nblocks=NBLK):
    """ins: dict of input APs (DRAM): ptsT [4,N], ptsS [128,384], w0..t3.
    out_final: DRAM [M, 1024]."""
    ptsT_in = ins["ptsT"]
    ptsS_in = ins["ptsS"]
    pts4_in = ins["pts4"]

    # ---------------- setup: points layouts ----------------
    T4 = sb.tile([32, N], F32)
    lhsTg = sb.tile([P, M], F32)
    lhsTgF = sb.tile([P, M], F32)
    ptsS = sb.tile([P, 3 * P], F32)
    X2 = sb.tile([P, P], F32)
    Y2 = sb.tile([P, P], F32)
    Z2 = sb.tile([P, P], F32)
    XSQ = sb.tile([P, P], F32)
    tmpPP = sb.tile([P, P], F32)

    if not scramble:
        # CoreSim flags uninitialized reads; on HW rows 4-31 are junk lanes
        # that feed transpose outputs nobody consumes, so skip the 22us memset.
        nc.gpsimd.memset(T4, 0.0)
    nc.sync.dma_start(out=T4[0:4, :], in_=ptsT_in)
    nc.sync.dma_start(out=ptsS, in_=ptsS_in)
    x, y, z = ptsS[:, 0:P], ptsS[:, P:2 * P], ptsS[:, 2 * P:3 * P]
    nc.vector.tensor_scalar(out=X2, in0=x, scalar1=-2.0, scalar2=None, op0=A.mult)
    nc.vector.tensor_scalar(out=Y2, in0=y, scalar1=-2.0, scalar2=None, op0=A.mult)
    nc.vector.tensor_scalar(out=Z2, in0=z, scalar1=-2.0, scalar2=None, op0=A.mult)
    nc.vector.tensor_tensor(out=XSQ, in0=x, in1=x, op=A.mult)
    nc.vector.tensor_tensor(out=tmpPP, in0=y, in1=y, op=A.mult)
    nc.vector.tensor_tensor(out=XSQ, in0=XSQ, in1=tmpPP, op=A.add)
    nc.vector.tensor_tensor(out=tmpPP, in0=z, in1=z, op=A.mult)
    nc.vector.tensor_tensor(out=XSQ, in0=XSQ, in1=tmpPP, op=A.add)
    nc.vector.tensor_scalar(out=tmpPP, in0=XSQ, scalar1=0.5, scalar2=None,
                            op0=A.mult)
    nc.sync.dma_start(out=T4[3:4, :].rearrange("a (p f) -> a p f", p=P),
                      in_=tmpPP)

    # ---------------- setup: weights (fold scale; transpose) -------------
    ident = sb.tile([P, P], F32)
    make_identity(nc, ident)
    w_e, b_f = {}, {}
    dims = {0: (64, 3), 1: (64, 64), 2: (128, 64), 3: (1024, 128)}
    # L0..L2: co<=128 so a single [co, ci] tile; L3: 8 blocks of [128, 128]
    for li, (co, ci) in dims.items():
        if li < 3:
            bf = sb.tile([co, 1], F32, name=f"bf{li}")
            b_f[li] = bf
            wsb = sb.tile([co, ci], F32, name=f"wsb{li}")
            ssb = sb.tile([co, 1], F32, name=f"ssb{li}")
            bsb = sb.tile([co, 1], F32, name=f"bsb{li}")
            tsb = sb.tile([co, 1], F32, name=f"tsb{li}")
            nc.sync.dma_start(out=wsb, in_=ins[f"w{li}"])
            nc.sync.dma_start(out=ssb, in_=ins[f"s{li}"][:, None])
            nc.sync.dma_start(out=bsb, in_=ins[f"b{li}"][:, None])
            nc.sync.dma_start(out=tsb, in_=ins[f"t{li}"][:, None])
            # wf = w * s ; bfold = b*s + t
            nc.vector.tensor_scalar(out=wsb, in0=wsb, scalar1=ssb,
                                    scalar2=None, op0=A.mult)
            nc.vector.scalar_tensor_tensor(out=b_f[li][:, 0:1], in0=bsb,
                                           scalar=ssb, in1=tsb,
                                           op0=A.mult, op1=A.add)
            # transpose to [ci, co]
            we = sb.tile([ci, co], F32, name=f"we{li}")
            wps = psum.tile([P, P], F32, name=f"wps{li}", tag="wps", bufs=1)
            nc.tensor.transpose(wps[0:ci, 0:co], wsb, ident[0:co, 0:co])
            nc.scalar.copy(out=we, in_=wps[0:ci, 0:co])
            w_e[li] = we
        else:
            # w3 [1024, 128]: fold+transpose per 128-row block -> we3 [128,1024]
            we = sb.tile([P, 1024], F32, name="we3")
            b3full = sb.tile([P, 8], F32, name="b3full")
            for cb in range(8):
                wsb = sb.tile([P, P], F32, name="wsb3", tag="wsb3")
                ssb = sb.tile([P, 1], F32, name="ssb3", tag="ssb3")
                bsb = sb.tile([P, 1], F32, name="bsb3", tag="bsb3")
                tsb = sb.tile([P, 1], F32, name="tsb3", tag="tsb3")
                sl = slice(cb * P, (cb + 1) * P)
                nc.sync.dma_start(out=wsb, in_=ins["w3"][sl, :])
                nc.sync.dma_start(out=ssb, in_=ins["s3"][sl, None])
                nc.sync.dma_start(out=bsb, in_=ins["b3"][sl, None])
                nc.sync.dma_start(out=tsb, in_=ins["t3"][sl, None])
                nc.vector.tensor_scalar(out=wsb, in0=wsb, scalar1=ssb,
                                        scalar2=None, op0=A.mult)
                nc.vector.scalar_tensor_tensor(
                    out=b3full[:, cb:cb + 1], in0=bsb, scalar=ssb, in1=tsb,
                    op0=A.mult, op1=A.add)
                wps = psum.tile([P, P], F32, name="wps3", tag="wps", bufs=1)
                nc.tensor.transpose(wps, wsb, ident)
                nc.scalar.copy(out=we[:, cb * P:(cb + 1) * P], in_=wps)
            w_e[3] = we
            b_f[3] = b3full

    # row offsets for index math
    rowoff_w = sb.tile([P, 1], I32)   # p * NW
    rowoff_c = sb.tile([P, 1], I32)   # p * 32
    nc.gpsimd.iota(rowoff_w, pattern=[[0, 1]], base=0, channel_multiplier=NW)
    nc.gpsimd.iota(rowoff_c, pattern=[[0, 1]], base=0, channel_multiplier=32)

    # ---------------- scan ----------------
    t = alloc_scan_tiles(sb)
    emit_scan_setup(nc, t, T4, lhsTg, M)
    with tc.tile_critical():
        emit_scan_loop(nc, t, T4, X2, Y2, Z2, XSQ, lhsTg, M)
    emit_scan_gather(nc, t, lhsTg, lhsTgF, None)

    nb_sv = nc.values_load(t["kinfo"][0:1, 1:2], min_val=1, max_val=NBLK) if dedup else None

    # ---------------- DRAM scratch ----------------
    scratch = dram.tile([M, 1024], F32)

    def offs_for(idx_ap, Q, name):
        """HW indirect DMA consumes offsets as offs[k%128, k//128] for dest
        slot k; pre-scramble so desc k sees idx.flat[k]. CoreSim ravels the
        AP directly, so no scramble there."""
        if not scramble:
            return idx_ap
        dtmp = dram.tile([P, Q], I32, name=f"scrd_{name}", tag=f"scrd_{name}")
        offs = sb.tile([P, Q], I32, name=f"scrs_{name}", tag=f"scrs_{name}")
        nc.sync.dma_start(out=dtmp, in_=idx_ap)
        nc.sync.dma_start(
            out=offs, in_=bass.AP(dtmp.tensor, dtmp.offset, [[1, P], [P, Q]]))
        return offs

    # static iota for the one-hot wid lookup: [128, r=32, s=32] value = s
    iotaS = sb.tile([P, R * R], I32)
    nc.gpsimd.iota(iotaS, pattern=[[0, R], [1, R]], base=0,
                   channel_multiplier=0)

    # ---------------- per-block tiles (shared across blocks) -------------
    lhsT_blk = sb.tile([32, P], F32)
    cblk = sb.tile([P, 32], F32)
    pooled = sb.tile([P, NW], F32)
    wv8 = sb.tile([P, 8], F32)
    wid = sb.tile([P, R], U32)
    cand4 = sb.tile([P, R * W * 4], F32)
    wid16 = sb.tile([P, R], mybir.dt.int16)
    widd = dram.tile([P, R], mybir.dt.int16)
    wtab = sb.tile([P, 256], mybir.dt.int16)
    dxt = sb.tile([P, NCAND], F32)
    sqt = sb.tile([P, NCAND], F32)
    candS = sb.tile([P, NCAND], F32)
    cv8 = sb.tile([P, 8], F32)
    candpos = sb.tile([P, R], U32)
    qsel = sb.tile([P, R], I32)
    onehot = sb.tile([P, R * R], I32)
    nsel = sb.tile([P, R], I32)
    nidx = sb.tile([P, R], I32)

    gacc_all = sb.tile([P, 8 * P], F32)
    rhs3g = sb.tile([4, P * R], F32)
    gre = sb.tile([P, P], F32)
    gT = sb.tile([P, 1024], F32)
    nc.vector.memset(lhsT_blk, 0.0)

    def block_body(bi, dyn):
        """bi: python int or ScalarValue block index."""
        base = bi * P
        # stage lhsT for this block (static-offset tile for matmul) and the
        # per-partition centroid coords via block transpose
        nc.vector.tensor_copy(out=lhsT_blk[0:4, :], in_=lhsTgF[0:4, ds(base, P)])
        for q in range(4):
            nc.vector.transpose(out=cblk[32 * q:32 * (q + 1), 0:32],
                                in_=lhsT_blk[0:32, 32 * q:32 * (q + 1)])
        # scores: 32 chunks of [128, 512]; pool windows of 16 from PSUM
        for ch in range(32):
            ps = psum.tile([P, 512], F32, name="score_ps", tag="score_ps",
                           bufs=2)
            nc.tensor.matmul(ps, lhsT_blk[0:4, :],
                             T4[0:4, ch * 512:(ch + 1) * 512],
                             start=True, stop=True)
            nc.vector.tensor_reduce(
                out=pooled[:, ch * 32:(ch + 1) * 32],
                in_=ps.rearrange("p (w e) -> p w e", e=W),
                axis=mybir.AxisListType.X, op=A.max)
        # top-32 windows
        for r in range(4):
            nc.vector.max(out=wv8, in_=pooled)
            nc.vector.max_index(out=wid[:, r * 8:(r + 1) * 8], in_max=wv8,
                                in_values=pooled)
            nc.vector.match_replace(out=pooled, in_to_replace=wv8,
                                    in_values=pooled, imm_value=NEG)
        if level == 1:
            return
        # flatten wid to one partition (defines HW+sim desc order) and gather
        # the candidate windows' points (256B descs from pts4 [16384, 4])
        # build wrapped+replicated i16 index table for dma_gather:
        # table[q, 8s + d] = wid[16d + q, s]; then replicate to 128 partitions
        nc.vector.tensor_copy(out=wid16, in_=wid.bitcast(I32))
        nc.sync.dma_start(out=widd, in_=wid16)
        nc.sync.dma_start(
            out=wtab[0:16, :],
            in_=bass.AP(widd.tensor, widd.offset,
                        [[32, 16], [1, 32], [512, 8]]))
        for rr in range(1, 8):
            nc.sync.dma_start(out=wtab[16 * rr:16 * (rr + 1), :],
                              in_=wtab[0:16, :])
        for qq in range(4):
            nc.gpsimd.dma_gather(
                out_ap=cand4.rearrange("p (w e) -> p w e", e=W * 4)[
                    :, 8 * qq:8 * (qq + 1), :],
                in_ap=pts4_in.rearrange("(a b) c -> a (b c)", b=W),
                idxs_ap=wtab[:, 64 * qq:64 * (qq + 1)],
                num_idxs=1024, num_idxs_reg=1024, elem_size=W * 4)
        # candidate scores = -(d^2), exact form (no cancellation)
        cx = cand4.rearrange("p (we c) -> p we c", c=4)
        for c in range(3):
            nc.vector.tensor_scalar(out=dxt, in0=cx[:, :, c],
                                    scalar1=cblk[:, c:c + 1], scalar2=None,
                                    op0=A.subtract)
            if c == 0:
                nc.vector.tensor_tensor(out=candS, in0=dxt, in1=dxt, op=A.mult)
            else:
                nc.vector.tensor_tensor(out=sqt, in0=dxt, in1=dxt, op=A.mult)
                nc.vector.tensor_tensor(out=candS, in0=candS, in1=sqt, op=A.add)
        nc.vector.tensor_scalar(out=candS, in0=candS, scalar1=-1.0,
                                scalar2=None, op0=A.mult)
        # top-32 candidates
        for r in range(4):
            nc.vector.max(out=cv8, in_=candS)
            nc.vector.max_index(out=candpos[:, r * 8:(r + 1) * 8], in_max=cv8,
                                in_values=candS)
            nc.vector.match_replace(out=candS, in_to_replace=cv8,
                                    in_values=candS, imm_value=NEG)
        if level == 2:
            return
        # widsel one-hot: nsel[p, r] = wid[p, candpos[p, r] >> 4]
        cpi = candpos.bitcast(I32)
        nc.vector.tensor_scalar(out=qsel, in0=cpi, scalar1=4, scalar2=None,
                                op0=A.logical_shift_right)
        nc.vector.tensor_tensor(
            out=onehot,
            in0=qsel[:, :, None].to_broadcast([P, R, R]),
            in1=iotaS.rearrange("p (r s) -> p r s", s=R), op=A.is_equal)
        nc.vector.tensor_tensor(
            out=onehot, in0=onehot,
            in1=wid.bitcast(I32)[:, None, :].to_broadcast([P, R, R]),
            op=A.mult)
        with nc.allow_low_precision(reason="int32 one-hot dot"):
            nc.vector.tensor_reduce(
                out=nsel, in_=onehot.rearrange("p (r s) -> p r s", s=R),
                axis=mybir.AxisListType.X, op=A.add)
        # n = nsel*16 + (candpos & 15)
        nc.vector.tensor_scalar(out=nsel, in0=nsel, scalar1=4, scalar2=None,
                                op0=A.logical_shift_left)
        nc.vector.tensor_scalar(out=nidx, in0=cpi, scalar1=15, scalar2=None,
                                op0=A.bitwise_and)
        nc.vector.tensor_tensor(out=nidx, in0=nidx, in1=nsel, op=A.add)
        nof = offs_for(nidx, R, "nidx")
        # gather member coords (x,y,z rows of ptsT)
        for c in range(3):
            nc.gpsimd.indirect_dma_start(
                out=rhs3g[c:c + 1, :].rearrange("a (j e) -> a j e", e=1),
                out_offset=None,
                in_=ptsT_in.rearrange("a n -> (a n)")[:, None],
                in_offset=IndirectOffsetOnAxis(ap=nof, axis=0),
                element_offset=c * N)
        if level == 3:
            return
        # ---- pointnet ----        if level == 3:
            return
        # ---- pointnet ----
        for ch in range(8):
            sl = slice(ch * 512, (ch + 1) * 512)
            h0c = sb.tile([64, 512], F32, name="h0c", tag="h0c", bufs=3)
            h1c = sb.tile([64, 512], F32, name="h1c", tag="h1c", bufs=3)
            h2c = sb.tile([P, 512], F32, name="h2c", tag="h2c", bufs=3)
            ps0 = psum.tile([64, 512], F32, name="mlp0", tag="mlpps", bufs=3)
            nc.tensor.matmul(ps0, w_e[0], rhs3g[0:3, sl], start=True, stop=True)
            nc.scalar.activation(out=h0c, in_=ps0, func=AF.Relu,
                                 bias=b_f[0][:, 0:1], scale=1.0)
            ps1 = psum.tile([64, 512], F32, name="mlp1", tag="mlpps", bufs=3)
            nc.tensor.matmul(ps1, w_e[1], h0c, start=True, stop=True)
            nc.scalar.activation(out=h1c, in_=ps1, func=AF.Relu,
                                 bias=b_f[1][:, 0:1], scale=1.0)
            ps2 = psum.tile([P, 512], F32, name="mlp2", tag="mlpps", bufs=3)
            nc.tensor.matmul(ps2, w_e[2], h1c, start=True, stop=True)
            nc.scalar.activation(out=h2c, in_=ps2, func=AF.Relu,
                                 bias=b_f[2][:, 0:1], scale=1.0)
            for cb in range(8):
                wsl = slice(cb * P, (cb + 1) * P)
                ps3 = psum.tile([P, 512], F32, name="mlp3", tag="mlp3", bufs=2)
                nc.tensor.matmul(ps3, w_e[3][:, wsl], h2c,
                                 start=True, stop=True)
                nc.vector.tensor_reduce(
                    out=gacc_all[:, cb * P + ch * 16:cb * P + (ch + 1) * 16],
                    in_=ps3.rearrange("p (g r) -> p g r", r=R),
                    axis=mybir.AxisListType.X, op=A.max)
        for cb in range(8):
            nc.scalar.activation(out=gre, in_=gacc_all[:, cb * P:(cb + 1) * P],
                                 func=AF.Relu, bias=b_f[3][:, cb:cb + 1],
                                 scale=1.0)
            pst = psum.tile([P, P], F32, name="gtp", tag="wps", bufs=1)
            nc.tensor.transpose(pst, gre, ident)
            nc.scalar.copy(out=gT[:, cb * P:(cb + 1) * P], in_=pst)
        # write block rows to scratch
        nc.sync.dma_start(out=scratch[ds(base, P), :], in_=gT)

    if dedup:
        tc.For_i_w_nested_ifs(start=0, end=nb_sv, step=1,
                              body=lambda bi: block_body(bi, True))
    else:
        for bi in range(nblocks):
            block_body(bi, False)

    # ---------------- output replication ----------------
    if level > 0:
        nc.sync.dma_start(out=out_final[0:P, 0:R],
                          in_=wid.bitcast(I32).bitcast(F32))
        return
    outSB = sb.tile([P, 4 * 1024], F32)
    nc.gpsimd.dma_gather(
        out_ap=outSB.rearrange("p (s c) -> p s c", s=4),
        in_ap=scratch[:], idxs_ap=t["idxs16"].bitcast(mybir.dt.int16),
        num_idxs=M, num_idxs_reg=M, elem_size=1024)
    nc.sync.dma_start(
        out=out_final.rearrange("(s p) c -> p s c", p=P),
        in_=outSB.rearrange("p (s c) -> p s c", s=4))


IN_KEYS = ["ptsT", "ptsS", "pts4", "w0", "b0", "s0", "t0",
           "w1", "b1", "s1", "t1", "w2", "b2", "s2", "t2",
           "w3", "b3", "s3", "t3"]
_CACHE = {}


def _host_scan_nblocks(points):
    """Replicates the device scan's exact fp32 decisions to determine how
    many 128-centroid blocks hold non-replicated centroids (the on-device
    output replication covers the rest). Worst case 4 (fully general)."""
    nb = 1
    for b in range(points.shape[0]):
        x = points[b, :, 0].copy()
        y = points[b, :, 1].copy()
        z = points[b, :, 2].copy()
        xsq = (x * x + y * y) + z * z
        n1, n2 = 0, -1
        k, go = 1, True
        while go and k < M:
            acc = (x * np.float32(-2) * x[n1] + xsq)
            acc = (y * np.float32(-2) * y[n1] + acc)
            acc = (z * np.float32(-2) * z[n1] + acc)
            am = acc.reshape(P, P)
            cm = am.max(axis=1)
            ci = am.argmax(axis=1)
            pk = ((cm.view(np.int32) & ~127) | ci.astype(np.int32)).view(
                np.float32)
            p = int(np.argmax(pk))
            n = p * 128 + int(pk.view(np.int32)[p] & 127)
            go = (n != n2)
            n2, n1 = n1, n
            k += 1
        kstar = k - 1 + (1 if go else 0)
        nb = max(nb, (kstar + 127) >> 7)
    return nb


def _build_nc(nblocks):
    nc = bacc.Bacc("TRN2", target_bir_lowering=False, debug=False,
                   enable_asserts=False, num_devices=NCORES)
    ins = {}
    ins["ptsT"] = nc.dram_tensor("ptsT", [4, N], F32, kind="ExternalInput").ap()
    ins["ptsS"] = nc.dram_tensor("ptsS", [P, 3 * P], F32,
                                 kind="ExternalInput").ap()
    ins["pts4"] = nc.dram_tensor("pts4", [N, 4], F32,
                                 kind="ExternalInput").ap()
    shapes = {"w0": [64, 3], "b0": [64], "s0": [64], "t0": [64],
              "w1": [64, 64], "b1": [64], "s1": [64], "t1": [64],
              "w2": [128, 64], "b2": [128], "s2": [128], "t2": [128],
              "w3": [1024, 128], "b3": [1024], "s3": [1024], "t3": [1024]}
    for k, shp in shapes.items():
        ins[k] = nc.dram_tensor(k, shp, F32, kind="ExternalInput").ap()
    out = nc.dram_tensor("out", [M, 1024], F32, kind="ExternalOutput").ap()
    with tile.TileContext(nc) as tc:
        with tc.tile_pool(name="sb", bufs=1) as sb, \
             tc.tile_pool(name="ps", bufs=1, space="PSUM") as psum, \
             tc.tile_pool(name="dr", bufs=1, space="DRAM") as dram:
            emit_kernel(tc, nc, sb, psum, dram, ins, out,
                        dedup=False, scramble=True, nblocks=nblocks)
    nc.compile()
    return nc


def kernel(**inputs):
    points = np.ascontiguousarray(inputs["points"], dtype=np.float32)
    B = points.shape[0]
    assert points.shape == (NCORES, N, 3)
    assert int(inputs["M"]) == M and int(inputs["R"]) == R
    nblocks = NBLK if os.environ.get("K_NODEDUP", "") == "1" else \
        _host_scan_nblocks(points)
    key = f"nc{nblocks}"
    if key not in _CACHE:
        _CACHE[key] = _build_nc(nblocks)
    nc = _CACHE[key]
    weights = {k: np.ascontiguousarray(inputs[k], dtype=np.float32)
               for k in IN_KEYS[3:]}
    in_maps = []
    for b in range(B):
        pts = points[b]
        ptsT = np.zeros((4, N), np.float32)
        ptsT[0:3, :] = pts.T
        ptsS = np.concatenate([pts[:, 0].reshape(P, P),
                               pts[:, 1].reshape(P, P),
                               pts[:, 2].reshape(P, P)], axis=1)
        pts4 = np.zeros((N, 4), np.float32)
        pts4[:, 0:3] = pts
        m = {"ptsT": ptsT, "ptsS": np.ascontiguousarray(ptsS), "pts4": pts4}
        m.update(weights)
        in_maps.append(m)
    res = run_bass_kernel_spmd(nc, in_maps, core_ids=list(range(NCORES)),
                               trace=os.environ.get("K_TRACE", "") == "1")
    out = np.stack([res.results[b]["out"] for b in range(B)], axis=0)
    _CACHE["last_results"] = res
    return out[..., None]

